# revision 5
# baseline (speedup 1.0000x reference)
"""KAN layer (LayerNorm -> per-bin Bernstein spline -> reduce over input dim)
as a Bass/Tile kernel for 8 trn2 NeuronCores.

Sharding: data-parallel over batch. Each core gets 8 rows of x and a full
copy of poly_matrix rearranged on the host to R[i*GRID+g, k*D_OUT+o] so the
coefficient block of one (batch, input, bin) triple is a single contiguous
DRAM row, gathered with indirect DMA.

Math per core (b in 0..7):
  ln/bins:  xn = LN(x); u = ((clip(xn)+1)*0.5)*GRID; g = floor(u); t = u-g
  basis:    bv[b,i,k] = Bernstein_k(t) via Horner with basis-matrix consts
  gather:   G[i, (k,o)] = R[i*GRID+g[b,i], :]   (one 8KB row per (b,i))
  contract: y[b,o] = sum_i sum_k bv[b,i,k] * G[i, k*D_OUT+o]
            -> PE matmuls, lhsT=bv column [128,1], rhs=G k-slice [128,512],
               accumulated in PSUM over 4 i-chunks x 4 k.
"""

import numpy as np

import concourse.bass as bass
import concourse.mybir as mybir
import concourse.tile as tile
from concourse import bacc
from concourse.bass_utils import run_bass_kernel_spmd
from concourse.masks import make_identity

B = 64          # total batch
D_IN = 512
D_OUT = 512
DEG = 3
GRID = 100
GRID_EPS = 1e-6
LN_EPS = 1e-5
N_CORES = 8
BPC = B // N_CORES          # batch rows per core (8)
NROWS = D_IN * GRID         # 51200 gatherable rows
ROW = (DEG + 1) * D_OUT     # 2048 elements per row (k-major, o-minor)
NCH = D_IN // 128           # 4 i-chunks of 128

F32 = mybir.dt.float32
I32 = mybir.dt.int32
AX = mybir.AxisListType
OP = mybir.AluOpType
AF = mybir.ActivationFunctionType

_CACHE = {}


def _build_nc(Mconst, apply_affine, poly_dt, repeat=1):
    """Build + compile the per-core Bass program.

    Mconst: 4x4 python floats of basis_matrix (power->Bernstein).
    apply_affine: apply ln_weight/ln_bias tiles (skipped when they are 1/0).
    poly_dt: mybir dtype of R / G / bv lhsT (float32 or bfloat16).
    repeat: unroll the gather+contract phase this many times (timing builds).
    """
    nc = bacc.Bacc("TRN2", target_bir_lowering=False, debug=False)

    x8 = nc.declare_dram_parameter("x8", [BPC, D_IN], F32, isOutput=False)
    R = nc.declare_dram_parameter("R", [NROWS, ROW], poly_dt, isOutput=False)
    if apply_affine:
        w8 = nc.declare_dram_parameter("w8", [BPC, D_IN], F32, isOutput=False)
        b8 = nc.declare_dram_parameter("b8", [BPC, D_IN], F32, isOutput=False)
    y8 = nc.declare_dram_parameter("y8", [BPC, D_OUT], F32, isOutput=True)

    with tile.TileContext(nc) as tc:
        with (
            tc.tile_pool(name="const", bufs=1) as cp,
            tc.tile_pool(name="work", bufs=1) as wp,
            tc.tile_pool(name="gpool", bufs=6) as gp,
            tc.tile_pool(name="outp", bufs=2) as op_,
            tc.tile_pool(name="ptr", bufs=2, space="PSUM") as ptr,
            tc.tile_pool(name="pacc", bufs=4, space="PSUM") as pacc,
        ):
            ident = cp.tile([128, 128], F32, tag="ident")
            make_identity(nc, ident[:])

            # row-base offsets i*GRID, as f32 (exact ints < 2^24)
            iotaI = cp.tile([BPC, D_IN], I32, tag="iotaI")
            nc.gpsimd.iota(iotaI[:], pattern=[[GRID, D_IN]], base=0,
                           channel_multiplier=0)
            iotaF = cp.tile([BPC, D_IN], F32, tag="iotaF")
            nc.vector.tensor_copy(iotaF[:], iotaI[:])

            x = wp.tile([BPC, D_IN], F32, tag="x")
            nc.sync.dma_start(x[:], x8[:])

            # ---- LayerNorm (two-pass, matching jnp.mean/jnp.var) ----
            sumx = wp.tile([BPC, 1], F32, tag="sumx")
            nc.vector.tensor_reduce(sumx[:], x[:], axis=AX.X, op=OP.add)
            mean = wp.tile([BPC, 1], F32, tag="mean")
            nc.vector.tensor_scalar_mul(mean[:], sumx[:], 1.0 / D_IN)
            xc = wp.tile([BPC, D_IN], F32, tag="xc")
            nc.vector.tensor_scalar(xc[:], x[:], mean[:, :1], None, OP.subtract)
            sq = wp.tile([BPC, D_IN], F32, tag="sq")
            nc.scalar.square(sq[:], xc[:])
            v = wp.tile([BPC, 1], F32, tag="v")
            nc.vector.tensor_reduce(v[:], sq[:], axis=AX.X, op=OP.add)
            # v = sumsq/D + eps
            nc.vector.tensor_scalar(v[:], v[:], 1.0 / D_IN, LN_EPS, OP.mult, OP.add)
            # rstd = rsqrt(v) via sqrt + reciprocal + one Newton step
            s = wp.tile([BPC, 1], F32, tag="s")
            nc.scalar.sqrt(s[:], v[:])
            r0 = wp.tile([BPC, 1], F32, tag="r0")
            nc.vector.reciprocal(r0[:], s[:])
            r2 = wp.tile([BPC, 1], F32, tag="r2")
            nc.vector.tensor_tensor(out=r2[:], in0=r0[:], in1=r0[:], op=OP.mult)
            nc.vector.tensor_tensor(out=r2[:], in0=r2[:], in1=v[:], op=OP.mult)
            nc.vector.tensor_scalar(r2[:], r2[:], -0.5, 1.5, OP.mult, OP.add)
            rstd = wp.tile([BPC, 1], F32, tag="rstd")
            nc.vector.tensor_tensor(out=rstd[:], in0=r0[:], in1=r2[:], op=OP.mult)

            xn = wp.tile([BPC, D_IN], F32, tag="xn")
            nc.vector.tensor_scalar(xn[:], xc[:], rstd[:, :1], None, OP.mult)
            if apply_affine:
                wt = wp.tile([BPC, D_IN], F32, tag="wt")
                bt = wp.tile([BPC, D_IN], F32, tag="bt")
                nc.sync.dma_start(wt[:], w8[:])
                nc.sync.dma_start(bt[:], b8[:])
                nc.vector.tensor_tensor(out=xn[:], in0=xn[:], in1=wt[:], op=OP.mult)
                nc.vector.tensor_tensor(out=xn[:], in0=xn[:], in1=bt[:], op=OP.add)

            # clip, map to [0, GRID)
            cl = wp.tile([BPC, D_IN], F32, tag="cl")
            nc.vector.tensor_scalar(cl[:], xn[:], -1.0 + GRID_EPS, 1.0 - GRID_EPS,
                                    OP.max, OP.min)
            # ((cl + 1) * 0.5) * 100  -- same op order as the reference
            u = wp.tile([BPC, D_IN], F32, tag="u")
            nc.vector.tensor_scalar(u[:], cl[:], 1.0, 0.5, OP.add, OP.mult)
            nc.vector.tensor_scalar(u[:], u[:], float(GRID), None, OP.mult)

            # floor(u) robust to either int-conversion rounding mode
            i1 = wp.tile([BPC, D_IN], I32, tag="i1")
            nc.vector.tensor_copy(i1[:], u[:])
            f1 = wp.tile([BPC, D_IN], F32, tag="f1")
            nc.vector.tensor_copy(f1[:], i1[:])
            gt = wp.tile([BPC, D_IN], F32, tag="gt")
            nc.vector.tensor_tensor(out=gt[:], in0=f1[:], in1=u[:], op=OP.is_gt)
            flr = wp.tile([BPC, D_IN], F32, tag="flr")
            nc.vector.tensor_tensor(out=flr[:], in0=f1[:], in1=gt[:], op=OP.subtract)
            t = wp.tile([BPC, D_IN], F32, tag="t")
            nc.vector.tensor_tensor(out=t[:], in0=u[:], in1=flr[:], op=OP.subtract)
            offsF = wp.tile([BPC, D_IN], F32, tag="offsF")
            nc.vector.tensor_tensor(out=offsF[:], in0=flr[:], in1=iotaF[:], op=OP.add)

            # ---- Bernstein basis via Horner ----
            bv = []
            for k in range(DEG + 1):
                m3, m2, m1, m0 = (Mconst[3][k], Mconst[2][k],
                                  Mconst[1][k], Mconst[0][k])
                h = wp.tile([BPC, D_IN], F32, tag=f"bv{k}")
                nc.scalar.activation(h[:], t[:], AF.Copy, bias=m2, scale=m3)
                nc.vector.tensor_tensor(out=h[:], in0=h[:], in1=t[:], op=OP.mult)
                nc.scalar.activation(h[:], h[:], AF.Copy, bias=m1, scale=1.0)
                nc.vector.tensor_tensor(out=h[:], in0=h[:], in1=t[:], op=OP.mult)
                nc.scalar.activation(h[:], h[:], AF.Copy, bias=m0, scale=1.0)
                bv.append(h)

            # ---- transpose offsets and bv to [128 i, BPC b] ----
            offsT = []
            bvT = [[None] * NCH for _ in range(DEG + 1)]
            for c in range(NCH):
                sl = slice(c * 128, (c + 1) * 128)
                pt = ptr.tile([128, BPC], F32, tag="ptr")
                nc.tensor.transpose(pt[:], offsF[:, sl], ident[:BPC, :BPC])
                ot = cp.tile([128, BPC], I32, tag=f"offsT{c}")
                nc.vector.tensor_copy(ot[:], pt[:])
                offsT.append(ot)
                for k in range(DEG + 1):
                    pb = ptr.tile([128, BPC], F32, tag="ptr")
                    nc.tensor.transpose(pb[:], bv[k][:, sl], ident[:BPC, :BPC])
                    bt_ = cp.tile([128, BPC], poly_dt, tag=f"bvT{k}_{c}")
                    nc.vector.tensor_copy(bt_[:], pb[:])
                    bvT[k][c] = bt_

            # ---- gather + contract ----
            for _rep in range(repeat):
              for b in range(BPC):
                    acc = pacc.tile([1, D_OUT], F32, tag="acc")
                    for c in range(NCH):
                        G = gp.tile([128, ROW], poly_dt, tag="G")
                        nc.gpsimd.indirect_dma_start(
                            out=G[:],
                            out_offset=None,
                            in_=R[:],
                            in_offset=bass.IndirectOffsetOnAxis(
                                ap=offsT[c][:, b:b + 1], axis=0),
                        )
                        for k in range(DEG + 1):
                            nc.tensor.matmul(
                                acc[:],
                                lhsT=bvT[k][c][:, b:b + 1],
                                rhs=G[:, k * D_OUT:(k + 1) * D_OUT],
                                start=(c == 0 and k == 0),
                                stop=(c == NCH - 1 and k == DEG),
                            )
                    orow = op_.tile([1, D_OUT], F32, tag="orow")
                    nc.vector.tensor_copy(orow[:], acc[:])
                    nc.sync.dma_start(y8[b:b + 1, :], orow[:])

    nc.compile()
    return nc


def _prep_R(poly_matrix, np_dt):
    # poly[i, o, g, k] -> R[i, g, k, o] -> [NROWS, ROW]
    R = np.ascontiguousarray(np.transpose(poly_matrix, (0, 2, 3, 1)))
    R = R.reshape(NROWS, ROW)
    if R.dtype != np_dt:
        R = R.astype(np_dt)
    return R


def get_compiled(basis_matrix, ln_weight, ln_bias, use_bf16=False, repeat=1):
    apply_affine = not (np.all(ln_weight == 1.0) and np.all(ln_bias == 0.0))
    Mkey = np.asarray(basis_matrix, np.float32).tobytes()
    key = (Mkey, apply_affine, use_bf16, repeat)
    if key not in _CACHE:
        Mconst = [[float(basis_matrix[j, k]) for k in range(DEG + 1)]
                  for j in range(DEG + 1)]
        poly_dt = mybir.dt.bfloat16 if use_bf16 else F32
        _CACHE[key] = _build_nc(Mconst, apply_affine, poly_dt, repeat)
    return _CACHE[key], apply_affine


USE_BF16 = False


def make_in_maps(x, poly_matrix, ln_weight, ln_bias, apply_affine, use_bf16):
    import ml_dtypes
    np_dt = ml_dtypes.bfloat16 if use_bf16 else np.float32
    R = _prep_R(np.asarray(poly_matrix), np_dt)
    x = np.asarray(x, np.float32)
    maps = []
    for c in range(N_CORES):
        m = {"x8": np.ascontiguousarray(x[c * BPC:(c + 1) * BPC]), "R": R}
        if apply_affine:
            m["w8"] = np.ascontiguousarray(
                np.broadcast_to(np.asarray(ln_weight, np.float32), (BPC, D_IN)))
            m["b8"] = np.ascontiguousarray(
                np.broadcast_to(np.asarray(ln_bias, np.float32), (BPC, D_IN)))
        maps.append(m)
    return maps


def kernel(x, poly_matrix, ln_weight, ln_bias, basis_matrix):
    nc, apply_affine = get_compiled(basis_matrix, ln_weight, ln_bias, USE_BF16)
    in_maps = make_in_maps(x, poly_matrix, ln_weight, ln_bias,
                           apply_affine, USE_BF16)
    res = run_bass_kernel_spmd(nc, in_maps, core_ids=list(range(N_CORES)))
    y = np.concatenate([res.results[c]["y8"] for c in range(N_CORES)], axis=0)
    return y.astype(np.float32)
